# revision 82
# baseline (speedup 1.0000x reference)
"""Trainium2 Bass kernel for nn_CrossAttention_45724221833727.

Data-parallel over batch: 8 samples -> 8 NeuronCores, one [S=2048, D=512]
cross-attention problem per core. Weights/pos replicated.

Every matmul runs fp8(e4m3)+DoubleRow (0.5 cyc/row, K=256 per
instruction -> 4x bf16 PE throughput); TimelineSim: 83.9us vs the 121.6us
bf16 baseline:
  - depthwise conv: taps DR-paired as (0,2),(1,3),(4,5) against a
    shift-packed fp8 copy of evo.T (4 shift rows per d-chunk keep every
    moving-AP base 4B aligned); stationaries are DR-packed diagonals
  - Q projection: DR over d-chunk pairs; pos+bq ride the PSUM->SBUF
    cast as a DVE tensor_tensor with fp8 posT
  - pointwise KV: DR over d-chunk pairs of the fp8 conv output. The pw
    bias is dropped from the scores path entirely (softmax over j is
    invariant to per-i constants); PV recovers it because sum(attn)=1,
    so out = PV0/l + kv0 + (evo + 2*pwb_eff), with the bias folded on
    host into the f32 residual tensor.
  - attention: fp8 DR scores/PV; row-sums via DR matmuls against a
    ones column; no max-subtract (|scores*scale| < ~2)

Scheduling (the in-order engine queues make emission order the
schedule):
  - all PSUM->SBUF casts read 2-bank-wide [P,2,512] PSUM tiles so each
    ACT/DVE op covers 1024 columns; a single rotating 3-slot PSUM pool
    serves conv/Q/KV matmul groups AND the kv/ke transposes
  - evo.T/pos stream per 512-column quarter (prefetched one quarter
    ahead); the serial DMA device round-robins queues, so the residual
    and output traffic are placed to never gate the critical loads
  - conv runs one quarter ahead of its consumer; scores+exp for an
    (i-tile, j-quarter) pair are woven one sub-block at a time into the
    NEXT quarter's emission (triangular prestart, ~56% of exp overlaps
    the projection phase); the rest weave between PV accumulation
    blocks, spaced >= the exp service time so the PE never stalls on
    the score-tile rotation
  - PV is ib-major so each 128-row output block finishes early and its
    epilogue chain (reciprocal -> DVE scale -> Pool/DVE residual add ->
    SP store) overlaps the remaining PV blocks

Error: the fp8 KV chain costs ~1.3e-2 max-rel vs the 2e-2 gate
(validated against an fp64 oracle in numpy, all 8 batches; the fp8
Q path is error-neutral and the residual keeps kv in bf16).
"""

import math

import numpy as np

import concourse.bass as bass
import concourse.mybir as mybir
import concourse.tile as tile
from concourse.bass_utils import run_bass_kernel_spmd
from concourse.masks import make_identity

F32 = mybir.dt.float32
BF16 = mybir.dt.bfloat16
FP8 = mybir.dt.float8e4
NP_BF16 = mybir.dt.np(mybir.dt.bfloat16)
NP_FP8 = mybir.dt.np(mybir.dt.float8e4)
DR = mybir.MatmulPerfMode.DoubleRow
P = 128
S = 2048
D = 512
KS = 6
N_CORES = 8
SB = S // P      # 16 s-blocks
DC = D // P      # 4 d-chunks
IT = S // 512    # 4 i-tiles of 512
PADW = 2056      # 2 left pad + 2048 + 3 right pad, rounded up
ACT_EXP = mybir.ActivationFunctionType.Exp
ACT_COPY = mybir.ActivationFunctionType.Copy
ADD = mybir.AluOpType.add
MULT = mybir.AluOpType.mult

_COMPILED = {}


def _install_tail_drain_patch():
    """This container's walrus build only accepts ONE sync wait per
    instruction; TileContext's tail drain carries one wait per live
    engine/DMA-queue. Split them across single-wait NOPs."""
    if getattr(tile.TileContext, "_tail_patch_installed", False):
        return

    def _patched_drain_and_barrier(self, tick_clock, wait_clock):
        from concourse.tile import ScopedClock

        drain_inst = self.nc.sync.drain()
        wait_clock.add_sem_waits(
            drain_inst.ins, ScopedClock({None: tick_clock.global_clock})
        )
        si = drain_inst.ins.sync_info
        waits = list(si.on_wait) if si and si.on_wait else []
        if len(waits) > 1:
            drain_inst.ins.sync_info = mybir.SyncInfo(
                on_wait=[], on_update=list(si.on_update or [])
            )
            for i, w in enumerate(waits):
                nop = self.nc.sync.nop(nofuse=True, hint=f"tail_wait_{i}")
                nop.ins.sync_info = mybir.SyncInfo(on_wait=[w], on_update=[])

        self.nc.all_engine_barrier()
        assert self.sems is not None
        popped = self.nc._tile_sem_poison_stack.pop()
        assert popped is self._sem_poison
        self.nc.clear_and_free_semaphores(list(self.sems.allocated().values()))
        self.nc.all_engine_barrier()

    tile.TileContext._drain_and_barrier = _patched_drain_and_barrier
    tile.TileContext._tail_patch_installed = True


def _split_multi_waits(nc):
    """Walrus in this container accepts at most ONE sync wait per
    instruction. Hoist extra waits onto single-wait NOPs inserted just
    before the instruction in the same engine's stream (equivalent
    semantics: the engine stalls at the NOP instead)."""
    ctr = [0]
    for fn in nc.m.functions:
        for blk in fn.blocks:
            insts = list(blk.instructions)
            out = []
            changed = False
            for inst in insts:
                si = inst.sync_info
                if si is not None and si.on_wait and len(si.on_wait) > 1:
                    waits = list(si.on_wait)
                    for w in waits[:-1]:
                        nop = mybir.InstNoOp(
                            name=f"splitw-{ctr[0]}", ins=[], outs=[]
                        )
                        ctr[0] += 1
                        nop.engine = inst.engine
                        nop.sync_info = mybir.SyncInfo(on_wait=[w], on_update=[])
                        out.append(nop)
                    inst.sync_info = mybir.SyncInfo(
                        on_wait=[waits[-1]], on_update=list(si.on_update or [])
                    )
                    changed = True
                out.append(inst)
            if changed:
                blk.instructions = out
    return nc


def _build(reps=1):
    _install_tail_drain_patch()
    nc = bass.Bass()
    # evot8: [128, 4dc * 4sh * PADW] fp8; row (dc*4+sh) = evo.T[dc-chunk]
    # shifted left by sh within the padded frame
    evot8_d = nc.dram_tensor("evot8", [P, DC * 4 * PADW], FP8,
                             kind="ExternalInput")
    # evo + 2*pwb_eff, f32, [S, D]  (residual, biases folded)
    evo_d = nc.dram_tensor("evo", [S, D], F32, kind="ExternalInput")
    # (pos.T + bq) fp8 [128, 4ec * S]
    post8_d = nc.dram_tensor("post8", [P, DC * S], FP8, kind="ExternalInput")
    # Wq.T DR-packed: [128, 2dcp * 2r * D]
    wq8_d = nc.dram_tensor("wq8", [P, 2 * 2 * D], FP8, kind="ExternalInput")
    # pw_w.T DR-packed: [128, 2dcp * 2r * D]
    pw8_d = nc.dram_tensor("pw8", [P, 2 * 2 * D], FP8, kind="ExternalInput")
    # conv diagonal stationaries DR-packed: [128, (dc*3+pair) * 2r * 128]
    dwdg8_d = nc.dram_tensor("dwdg8", [P, DC * 3 * 2 * P], FP8,
                             kind="ExternalInput")
    out_d = nc.dram_tensor("out", [S, D], F32, kind="ExternalOutput")

    with tile.TileContext(nc) as tc:
        for rep in range(reps):
            _emit_body(nc, tc, evot8_d, evo_d, post8_d, wq8_d, pw8_d,
                       dwdg8_d, out_d, rep)

    _split_multi_waits(nc)
    return nc


def _emit_body(nc, tc, evot8_d, evo_d, post8_d, wq8_d, pw8_d, dwdg8_d,
               out_d, rep):
    scale = 1.0 / math.sqrt(float(D))
    evo_r = evo_d.rearrange("(x p) d -> p x d", p=P)
    out_r = out_d.rearrange("(x p) d -> p x d", p=P)

    def pool(name, **kw):
        return tc.alloc_tile_pool(name=f"{name}r{rep}", **kw)

    cpool = pool("consts", bufs=1, side="left")

    # ---- constants. DMA queues: SP carries evot8 chunks (in consumption
    # order; the queue sem is coarse), ACT carries the small weights, DVE
    # carries the f32 residual quarters, so nothing big serializes ahead
    # of the first conv matmul.
    identb = cpool.tile([P, P], BF16, tag="identb")
    make_identity(nc, identb)
    ones_col = cpool.tile([P, 2, 2], FP8, tag="ones_col")
    nc.vector.memset(ones_col[:], 1.0)
    dwdg8 = cpool.tile([P, DC * 3, 2, P], FP8, tag="dwdg8")
    nc.scalar.dma_start(out=dwdg8[:], in_=dwdg8_d[:, :])
    wq8 = cpool.tile([P, 2, 2, D], FP8, tag="wq8")
    nc.scalar.dma_start(out=wq8[:], in_=wq8_d[:, :])
    pw8 = cpool.tile([P, 2, 2, D], FP8, tag="pw8")
    nc.scalar.dma_start(out=pw8[:], in_=pw8_d[:, :])

    # evoT8 / posT8 stream per quarter (column slices, prefetched one
    # quarter ahead) so the serial DMA device never gates compute
    QW = 520  # 512 + 5 conv taps, padded to 8
    etpool = pool("evoT", bufs=2, side="right")
    evot8_v = evot8_d.rearrange("p (r x) -> p r x", x=PADW)
    post8_v = post8_d.rearrange("p (r x) -> p r x", x=S)

    def load_quarter(q, split=False):
        ev = etpool.tile([P, DC * 4, QW], FP8, tag="evoT8q", name="evoT8q")
        if split:
            # first quarter: halve the transfer so conv(dc0,1) starts
            # as soon as the first half lands
            nc.sync.dma_start(out=ev[:, 0:8, :],
                              in_=evot8_v[:, 0:8, q * 512:q * 512 + QW])
            nc.sync.dma_start(out=ev[:, 8:16, :],
                              in_=evot8_v[:, 8:16, q * 512:q * 512 + QW])
        else:
            nc.sync.dma_start(out=ev[:],
                              in_=evot8_v[:, :, q * 512:q * 512 + QW])
        po = etpool.tile([P, DC, 512], FP8, tag="post8q", name="post8q")
        nc.scalar.dma_start(out=po[:], in_=post8_v[:, :, q * 512:(q + 1) * 512])
        return ev, po

    # ---- persistent activation stores ----
    qpool = pool("qt", bufs=1, side="left")
    qt8 = [qpool.tile([P, 2, S], FP8, tag=f"qt{pp}", name=f"qt{pp}")
           for pp in range(2)]
    kvtpool = pool("kvt", bufs=1, side="left")
    kvt8 = [kvtpool.tile([P, 2, S], FP8, tag=f"kvt{pp}", name=f"kvt{pp}")
            for pp in range(2)]
    kvtb = kvtpool.tile([P, 2, 2, S], BF16, tag="kvtb", name="kvtb")
    kvpool = pool("kv", bufs=1, side="left")
    kv8 = [kvpool.tile([P, 2, D], FP8, tag=f"kv{m}", name=f"kv{m}")
           for m in range(SB // 2)]
    kepool = pool("ke", bufs=1, side="left")
    ke_all = kepool.tile([P, SB, D], F32, tag="ke", name="ke")
    # all p8 tiles persist until PV (exp is prestarted during projections)
    ppool = pool("p8", bufs=1, side="left")
    p8 = [[ppool.tile([P, 2, 512], FP8, tag=f"p8_{ig}_{m}",
                      name=f"p8_{ig}_{m}")
           for m in range(SB // 2)] for ig in range(IT)]
    kvdwpool = pool("kvdw", bufs=2, side="right")
    evqpool = pool("evq", bufs=2, side="right")

    # ---- PSUM pools (8 banks):
    # B-D: mps 2x2 banks (matmul groups AND transposes share the tag) +
    #      pssw 2x2 (wide score tiles -> 1024-col exps) = 8
    # E:   pso 5 + psl 1 + pss 2 = 8 (B-D pools released first)
    pss = pool("pss", bufs=2, space="PSUM")
    mps = pool("mps", bufs=3, space="PSUM")

    # conv tap pairs, all 4B-aligned (within the quarter slice view
    # "p dc s2 par x" where row sh = 2*s2 + par):
    #   pair 0 = taps (0,2): rows (sh0, sh2) = par 0, s2 0..1, base 0
    #   pair 1 = taps (1,3): rows (sh1, sh3) = par 1, s2 0..1, base 0
    #   pair 2 = taps (4,5): rows (sh0, sh1) = s2 0, par 0..1, base 4

    def emit_conv(q, ev):
        # depthwise conv (fp8 DR), wide pairs of d-chunks; runs one
        # quarter ahead of its consumer (KVT)
        evv = ev.rearrange("p (dc s2 par) x -> p dc s2 par x", s2=2, par=2)

        def conv_mv(dc, pair):
            if pair == 0:
                return evv[:, dc, :, 0, 0:512]
            if pair == 1:
                return evv[:, dc, :, 1, 0:512]
            return evv[:, dc, 0, :, 4:516]

        kvdw_q = kvdwpool.tile([P, 2, 2, 512], FP8, tag="kvdwq", name="kvdwq")
        for dcp in range(2):
            ps = mps.tile([P, 2, 512], F32, tag="mm", name="cps")
            for half in range(2):
                dc = 2 * dcp + half
                for pair in range(3):
                    st = dwdg8[:, dc * 3 + pair, :, :]
                    nc.tensor.matmul(ps[:, half, :], st, conv_mv(dc, pair),
                                     start=(pair == 0), stop=(pair == 2),
                                     perf_mode=DR)
            nc.scalar.copy(kvdw_q[:, dcp, :, :], ps[:])  # ACT, 1024 wide
        return kvdw_q

    def emit_pair(ig, jq):
        # scores + exp for i-tile ig against j-quarter jq (4 jb blocks)
        for jb in range(4 * jq, 4 * jq + 4):
            s_ps = pss.tile([P, 512], F32, tag="sps", name="sps")
            for pp in range(2):
                nc.tensor.matmul(
                    s_ps[:], kvt8[pp][:, :, jb * P:(jb + 1) * P],
                    qt8[pp][:, :, ig * 512:(ig + 1) * 512],
                    start=(pp == 0), stop=(pp == 1), perf_mode=DR,
                )
            nc.scalar.activation(p8[ig][jb // 2][:, jb % 2, :], s_ps[:],
                                 ACT_EXP, scale=scale)

    def emit_quarter(q, cur, nxt, kvdw_q, fills):
        # sub-blocks with prestart pairs (from quarter q-1) interleaved so
        # no engine's in-order queue gets walled off behind a block
        ev, po = cur

        def fill():
            if fills:
                emit_pair(*fills.pop(0))

        # --- Q projection (fp8 DR); pos+bq ride the DVE cast ---
        # (first on PE in quarter 0 so DVE's first cast comes early)
        evv = ev.rearrange("p (dc s2 par) x -> p dc s2 par x", s2=2, par=2)
        it = q
        for pp in range(2):
            ps = mps.tile([P, 2, 512], F32, tag="mm", name="qps")
            for half in range(2):
                ec = 2 * pp + half
                for dcp in range(2):
                    nc.tensor.matmul(
                        ps[:, half, :], wq8[:, dcp, :, ec * P:(ec + 1) * P],
                        evv[:, 2 * dcp:2 * dcp + 2, 1, 0, 0:512],
                        start=(dcp == 0), stop=(dcp == 1), perf_mode=DR,
                    )
            nc.vector.tensor_tensor(
                qt8[pp][:, :, it * 512:(it + 1) * 512], ps[:],
                po[:, 2 * pp:2 * pp + 2, :], ADD,
            )
            fill()
        # prefetch next quarter's slices, then this quarter's residual,
        # in SP-queue order behind the already-critical loads
        if nxt is not None:
            nxt.append(load_quarter(q + 1))
        evoq = evqpool.tile([P, 4, 512], F32, tag="evq", name="evq")
        nc.sync.dma_start(out=evoq[:], in_=evo_r[:, q * 4:(q + 1) * 4, :])

        # --- pointwise KV (fp8 DR), no bias (softmax-invariant) ---
        for pp in range(2):
            ps = mps.tile([P, 2, 512], F32, tag="mm", name="dps")
            for half in range(2):
                ob = 2 * pp + half
                for dcp in range(2):
                    nc.tensor.matmul(
                        ps[:, half, :], pw8[:, dcp, :, ob * P:(ob + 1) * P],
                        kvdw_q[:, dcp, :, :],
                        start=(dcp == 0), stop=(dcp == 1), perf_mode=DR,
                    )
            nc.vector.tensor_copy(
                kvt8[pp][:, :, q * 512:(q + 1) * 512], ps[:])
            # kvtb alternates ACT/DVE for per-quarter balance
            if (q + pp) % 2 == 0:
                nc.scalar.copy(kvtb[:, pp, :, q * 512:(q + 1) * 512], ps[:])
            else:
                nc.vector.tensor_copy(
                    kvtb[:, pp, :, q * 512:(q + 1) * 512], ps[:])
            fill()

        # next quarter's conv, behind this quarter's matmuls on PE
        nxt_kvdw = None
        if q < 3:
            nxt_kvdw = emit_conv(q + 1, nxt[-1][0] if nxt else cur[0])
            fill()

        # --- kv8 (fp8) and ke (f32 residual) via bf16 transposes ---
        for m in range(2 * q, 2 * q + 2):
            tpw = mps.tile([P, 2, 512], BF16, tag="mm", name="tpw")
            for r in range(2):
                sb = 2 * m + r
                for dc in range(DC):
                    nc.tensor.transpose(
                        tpw[:, r, dc * P:(dc + 1) * P],
                        kvtb[:, dc // 2, dc % 2, sb * P:(sb + 1) * P],
                        identb[:],
                    )
            if m % 2 == 0:                                   # wide, alt
                nc.scalar.copy(kv8[m][:], tpw[:])
            else:
                nc.vector.tensor_copy(kv8[m][:], tpw[:])
            nc.vector.tensor_tensor(                         # DVE, wide
                ke_all[:, 2 * m:2 * m + 2, :], tpw[:],
                evoq[:, (2 * m) % 4:(2 * m) % 4 + 2, :], ADD,
            )
            fill()
        while fills:
            emit_pair(*fills.pop(0))
        return nxt_kvdw

    # ---- phases B-D with triangular prestart of scores/exp ----
    slices_q = [load_quarter(0, split=True)]
    kvdw_cur = emit_conv(0, slices_q[0][0])
    fills = []
    for q in range(4):
        kvdw_cur = emit_quarter(q, slices_q[q],
                                slices_q if q < 3 else None,
                                kvdw_cur, fills)
        fills = ([(ig, q) for ig in range(q)]
                 + [(q, jq) for jq in range(q + 1)])

    evqpool.release()
    kvdwpool.release()
    mps.release()
    pss.release()
    etpool.release()

    # ---- phase E: PV + epilogue, with quarter-3's 7 prestart pairs
    # interleaved between PV accumulation blocks. PV is ib-major (one
    # accumulation live at a time), so pso=3 suffices and the score
    # tiles go 2-bank wide: each E exp covers 1024 columns.
    pso = pool("pso", bufs=3, space="PSUM")
    psl = pool("psl", bufs=1, space="PSUM")
    pssw = pool("pssw", bufs=2, space="PSUM")
    epipool = pool("epi", bufs=2, side="right")
    rlpool = pool("rl", bufs=4, side="right")

    l_ps = psl.tile([P, IT, 2 * DC], F32, tag="lps", name="lps")

    out_r = out_d.rearrange("(x p) d -> p x d", p=P)

    def emit_epilogue(ig, out_ps, rl_sb):
        # per-block streaming: scale, residual add, narrow DMA out.
        # Steady state: scales on DVE (ACT is exp-bound), adds on Pool.
        # Last i-tile: engines alternate + DMAs split across queues so
        # the final chain is as short as possible.
        last = ig == IT - 1
        o_sb = epipool.tile([P, 4, 512], F32, tag="osb", name="osb")
        for ib in range(4):
            sb = ig * 4 + ib
            if last and ib % 2 == 1:
                nc.scalar.activation(
                    o_sb[:, ib, :], out_ps[ib][:], ACT_COPY,
                    scale=rl_sb[:, ib:ib + 1],
                )
                nc.vector.tensor_tensor(o_sb[:, ib, :], o_sb[:, ib, :],
                                        ke_all[:, sb, :], ADD)
                nc.scalar.dma_start(out=out_r[:, sb, :], in_=o_sb[:, ib, :])
            else:
                nc.vector.tensor_scalar(
                    o_sb[:, ib, :], out_ps[ib][:], rl_sb[:, ib:ib + 1], None,
                    MULT,
                )
                nc.gpsimd.tensor_tensor(o_sb[:, ib, :], o_sb[:, ib, :],
                                        ke_all[:, sb, :], ADD)
                nc.sync.dma_start(out=out_r[:, sb, :], in_=o_sb[:, ib, :])

    def emit_m(ig, m):
        # one m-pair: two score blocks into a 2-bank tile, one wide exp
        s_ps = pssw.tile([P, 2, 512], F32, tag="spw", name="spw")
        for r in range(2):
            jb = 2 * m + r
            for pp in range(2):
                nc.tensor.matmul(
                    s_ps[:, r, :], kvt8[pp][:, :, jb * P:(jb + 1) * P],
                    qt8[pp][:, :, ig * 512:(ig + 1) * 512],
                    start=(pp == 0), stop=(pp == 1), perf_mode=DR,
                )
        nc.scalar.activation(p8[ig][m][:, :, :], s_ps[:],
                             ACT_EXP, scale=scale)

    # dissolve the remaining pairs into an m-pair stream: pair (ig,3)
    # is emitted just before PV(ig) (its tiles are read by every PV(ig)
    # block); the rest spread across earlier PV slots so exp latency
    # hides behind PV matmuls without stalling the in-order PE queue
    m_stream = [(ig_, m) for (ig_, jq) in fills if ig_ != 0
                for m in (2 * jq, 2 * jq + 1)]
    js = 0

    def emit_pv_ib(ig, ib, out_ps):
        # ib-major: each output block's accumulation completes early so
        # its epilogue chain overlaps the remaining PV blocks
        nonlocal js
        for m in range(SB // 2):
            pm = p8[ig][m]
            nc.tensor.matmul(
                out_ps[:], pm[:, :, ib * P:(ib + 1) * P], kv8[m][:],
                start=(m == 0), stop=(m == SB // 2 - 1), perf_mode=DR,
            )
            nc.tensor.matmul(
                l_ps[:, ig, 2 * ib:2 * ib + 2],
                pm[:, :, ib * P:(ib + 1) * P], ones_col[:],
                start=(m == 0), stop=(m == SB // 2 - 1), perf_mode=DR,
            )
            if m in (0, 3, 6) and js < len(m_stream):
                emit_m(*m_stream[js])
                js += 1

    emit_m(0, 6)   # pair (0,3): PV(0) prelude
    emit_m(0, 7)
    for ig in range(IT):
        last = ig == IT - 1
        o_sb = epipool.tile([P, 4, 512], F32, tag="osb", name="osb")
        rl_sb = rlpool.tile([P, DC], F32, tag="rls", name="rls")
        for ib in range(4):
            out_ps = pso.tile([P, 512], F32, tag="ops", name="ops")
            emit_pv_ib(ig, ib, out_ps)
            sb = ig * 4 + ib
            nc.vector.reciprocal(rl_sb[:, ib:ib + 1],
                                 l_ps[:, ig, 2 * ib:2 * ib + 1])
            # scale on DVE (ACT's in-order queue is exp-saturated in E,
            # except at the very tail where it has drained); residual
            # adds alternate Pool/DVE; stores on the SP queue
            if last and ib % 2 == 1:
                nc.scalar.activation(o_sb[:, ib, :], out_ps[:], ACT_COPY,
                                     scale=rl_sb[:, ib:ib + 1])
            else:
                nc.vector.tensor_scalar(
                    o_sb[:, ib, :], out_ps[:], rl_sb[:, ib:ib + 1], None,
                    MULT,
                )
            if ib % 2 == 0:
                nc.gpsimd.tensor_tensor(o_sb[:, ib, :], o_sb[:, ib, :],
                                        ke_all[:, sb, :], ADD)
            else:
                nc.vector.tensor_tensor(o_sb[:, ib, :], o_sb[:, ib, :],
                                        ke_all[:, sb, :], ADD)
            nc.sync.dma_start(out=out_r[:, sb, :], in_=o_sb[:, ib, :])

    rlpool.release()
    epipool.release()
    pssw.release()
    psl.release()
    pso.release()
    ppool.release()
    kepool.release()
    kvpool.release()
    kvtpool.release()
    qpool.release()
    cpool.release()


def prep_in_maps(evo_local, Wq, bq, dw_w, dw_b, pw_w, pw_b, pos):
    evo_local = np.asarray(evo_local, dtype=np.float32)
    Wq = np.asarray(Wq, dtype=np.float32)
    bq = np.asarray(bq, dtype=np.float32)
    dw_w = np.asarray(dw_w, dtype=np.float32)
    dw_b = np.asarray(dw_b, dtype=np.float32)
    pw_w = np.asarray(pw_w, dtype=np.float32)
    pw_b = np.asarray(pw_b, dtype=np.float32)
    pos = np.asarray(pos, dtype=np.float32)

    pwb_eff = (pw_b + pw_w @ dw_b).astype(np.float32)         # fold dw bias
    post_eff = (pos[0].T + bq[:, None])                       # [d, s]
    post8 = np.ascontiguousarray(
        post_eff.reshape(DC, P, S).transpose(1, 0, 2)).astype(NP_FP8)
    post8 = post8.reshape(P, DC * S)
    wqt = Wq.T                                                # [d, e]
    wq8 = np.ascontiguousarray(
        wqt.reshape(2, 2, P, D).transpose(2, 0, 1, 3)).astype(NP_FP8)
    wq8 = wq8.reshape(P, 2 * 2 * D)
    pwt = pw_w.T                                              # [d, o]
    pw8 = np.ascontiguousarray(
        pwt.reshape(2, 2, P, D).transpose(2, 0, 1, 3)).astype(NP_FP8)
    pw8 = pw8.reshape(P, 2 * 2 * D)
    dww = dw_w[:, 0, :]                                       # [D, K]
    # tap pairing must match the kernel's aligned moving APs
    TAPS = [(0, 2), (1, 3), (4, 5)]
    dwdg8 = np.zeros((P, DC * 3, 2, P), np.float32)
    for dc in range(DC):
        for pair in range(3):
            for r in range(2):
                k = TAPS[pair][r]
                dwdg8[np.arange(P), dc * 3 + pair, r, np.arange(P)] = \
                    dww[dc * P:(dc + 1) * P, k]
    dwdg8 = dwdg8.astype(NP_FP8).reshape(P, DC * 3 * 2 * P)

    in_maps = []
    for c in range(N_CORES):
        evoT_pad = np.zeros((D, PADW + 3), np.float32)
        evoT_pad[:, 2:2 + S] = evo_local[c].T
        ev8 = np.zeros((P, DC, 4, PADW), np.float32)
        for dc in range(DC):
            for sh in range(4):
                ev8[:, dc, sh, :] = evoT_pad[dc * P:(dc + 1) * P,
                                             sh:sh + PADW]
        ev8 = ev8.astype(NP_FP8).reshape(P, DC * 4 * PADW)
        evo_res = (evo_local[c] + 2.0 * pwb_eff[None, :]).astype(np.float32)
        in_maps.append({
            "evot8": ev8,
            "evo": np.ascontiguousarray(evo_res),
            "post8": post8,
            "wq8": wq8,
            "pw8": pw8,
            "dwdg8": dwdg8,
        })
    return in_maps


def kernel(evo_local, Wq, bq, dw_w, dw_b, pw_w, pw_b, pos):
    in_maps = prep_in_maps(evo_local, Wq, bq, dw_w, dw_b, pw_w, pw_b, pos)
    if "nc" not in _COMPILED:
        _COMPILED["nc"] = _build()
    nc = _COMPILED["nc"]
    res = run_bass_kernel_spmd(nc, in_maps, core_ids=list(range(N_CORES)))
    out = np.stack([res.results[c]["out"] for c in range(N_CORES)], axis=0)
    return out.astype(np.float32)
